# revision 4
# baseline (speedup 1.0000x reference)
"""Trainium2 Bass/Tile kernel: batched dot-product attention with length masking.

Problem: queries/keys/values [32, 1024, 128] f32, valid_length [32] int64.
  out = softmax(mask(Q K^T / sqrt(128))) @ V

Strategy:
  - Data-parallel: 32 batches sharded 4-per-core across 8 NeuronCores (SPMD,
    identical program, per-core input maps).
  - Host prep per batch:
      qT/kT = Q^T/K^T  [128=D, 1024]  (contraction dim on SBUF partitions)
      vaug  = [V * rowmask, rowmask]  [1024, 129] bf16 (mask folded into V; the
              extra column makes the PV matmul also produce the softmax
              denominator)
  - Device per batch:
      S^T[k, q] = (K^T).T @ (Q^T)   fp32r matmuls (full PE rate, ~fp32 accuracy)
      P^T = exp(S^T * 1/sqrt(D))    ScalarE, PSUM->SBUF, bf16 out.  No rowmax
                                    needed: scores ~ N(0,1), |S| <~ 6.
      O_aug[q, 0:129] = sum_kb (P^T_kb).T @ Vaug_kb   bf16 matmuls, PSUM accum
      out[q, :] = O_aug[q, 0:128] * (1 / O_aug[q, 128])
  - Length specialization: batches sorted by valid_length desc and assigned
    round-robin so slot j on every core has a similar length; the program is
    compiled per (kb_counts) with fully-masked k-blocks skipped. Mask columns
    inside a partial block are handled by the vaug masking.
"""

import os

import numpy as np
import ml_dtypes

import concourse.tile as tile
from concourse import bacc, mybir
from concourse.bass_utils import run_bass_kernel_spmd

B, Q, K, D = 32, 1024, 1024, 128
N_CORES = 8
BPC = B // N_CORES  # batches per core
KB_MAX = K // 128
QH = 512  # moving-operand chunk for the S^T matmul (fp32 max)
SCALE = float(1.0 / np.sqrt(D))

# S-matmul operand mode: "f32r" (fp32 data, reduced-precision full-rate PE),
# "f32" (exact, 4 cyc/row), "bf16" (half DMA, ~1e-2 err)
S_DTYPE = os.environ.get("ATTN_S_DTYPE", "f32r")
# Disable the per-valid-length program specialization (all 8 k-blocks always)
NO_SPECIALIZE = os.environ.get("ATTN_NO_SPECIALIZE", "0") == "1"

LAST_RESULTS = None  # test harness introspection: last BassKernelResults
_NC_CACHE: dict = {}


def _body(tc, qT, kT, vaug, out, kb_counts, sdt):
    nc = tc.nc
    f32 = mybir.dt.float32
    bf16 = mybir.dt.bfloat16
    f32r = mybir.dt.float32r
    AF = mybir.ActivationFunctionType

    with (
        tc.tile_pool(name="qk", bufs=2) as qk_pool,
        tc.tile_pool(name="v", bufs=2) as v_pool,
        tc.tile_pool(name="p", bufs=2) as p_pool,
        tc.tile_pool(name="osb", bufs=4) as osb_pool,
        tc.tile_pool(name="den", bufs=4) as den_pool,
        tc.tile_pool(name="spsum", bufs=2, space="PSUM") as s_pool,
        tc.tile_pool(name="opsum", bufs=4, space="PSUM") as o_pool,
    ):
        qk_dt = {"bf16": bf16, "f32r": f32r, "f32": f32}[sdt]
        for b in range(BPC):
            KB = kb_counts[b]
            KC = KB * 128

            q_sb = qk_pool.tile([128, Q], qk_dt, tag="q")
            nc.sync.dma_start(out=q_sb[:], in_=qT[b])
            k_sb = qk_pool.tile([128, KC], qk_dt, tag="k")
            nc.sync.dma_start(out=k_sb[:], in_=kT[b][:, 0:KC])
            v_tiles = []
            for kb in range(KB):
                v_t = v_pool.tile([128, D + 1], bf16, tag=f"v{kb}")
                nc.sync.dma_start(out=v_t[:], in_=vaug[b, kb])
                v_tiles.append(v_t)

            # S^T[k, q] per k-block, then P^T = exp(scale * S^T) in bf16
            p_tiles = []
            for kb in range(KB):
                s_ps = s_pool.tile([128, Q], f32)
                lhsT = k_sb[:, kb * 128 : (kb + 1) * 128]
                for qh in range(Q // QH):
                    rhs = q_sb[:, qh * QH : (qh + 1) * QH]
                    nc.tensor.matmul(
                        s_ps[:, qh * QH : (qh + 1) * QH],
                        lhsT,
                        rhs,
                        start=True,
                        stop=True,
                    )
                p_t = p_pool.tile([128, Q], bf16, tag=f"p{kb}")
                nc.scalar.activation(p_t[:], s_ps[:], AF.Exp, scale=SCALE)
                p_tiles.append(p_t)

            # O_aug[q, 0:129] accumulated over k-blocks; col 128 = denominator
            for qb in range(Q // 128):
                o_ps = o_pool.tile([128, D + 1], f32)
                for kb in range(KB):
                    nc.tensor.matmul(
                        o_ps[:],
                        p_tiles[kb][:, qb * 128 : (qb + 1) * 128],
                        v_tiles[kb][:],
                        start=(kb == 0),
                        stop=(kb == KB - 1),
                    )
                den = den_pool.tile([128, 1], f32)
                nc.vector.reciprocal(den[:], o_ps[:, D : D + 1])
                o_sb = osb_pool.tile([128, D], f32)
                nc.scalar.activation(o_sb[:], o_ps[:, 0:D], AF.Copy, scale=den[:])
                nc.sync.dma_start(
                    out=out[b][qb * 128 : (qb + 1) * 128, :], in_=o_sb[:]
                )


def _build(kb_counts, sdt):
    key = (tuple(kb_counts), sdt)
    if key in _NC_CACHE:
        return _NC_CACHE[key]
    nc = bacc.Bacc("TRN2", target_bir_lowering=False, debug=False,
                   enable_asserts=False)
    f32 = mybir.dt.float32
    bf16 = mybir.dt.bfloat16
    qk_dt = {"bf16": bf16, "f32r": mybir.dt.float32r, "f32": f32}[sdt]
    qT = nc.dram_tensor("qT", [BPC, D, Q], qk_dt, kind="ExternalInput").ap()
    kT = nc.dram_tensor("kT", [BPC, D, K], qk_dt, kind="ExternalInput").ap()
    vaug = nc.dram_tensor(
        "vaug", [BPC, KB_MAX, 128, D + 1], bf16, kind="ExternalInput"
    ).ap()
    out = nc.dram_tensor("out", [BPC, Q, D], f32, kind="ExternalOutput").ap()
    with tile.TileContext(nc) as tc:
        _body(tc, qT, kT, vaug, out, kb_counts, sdt)
    nc.compile()
    _NC_CACHE[key] = nc
    return nc


def _prep(queries, keys, values, valid_length):
    """Returns (in_maps, assign, kb_counts). assign[j, c] = original batch index
    handled by core c slot j."""
    vl = np.asarray(valid_length).astype(np.int64).reshape(B)
    if NO_SPECIALIZE:
        assign = np.arange(B).reshape(N_CORES, BPC).T  # core-major, no sort
        kb_counts = tuple([KB_MAX] * BPC)
    else:
        order = np.argsort(-vl, kind="stable")
        assign = order.reshape(BPC, N_CORES)  # [slot, core]
        kb_counts = tuple(
            max(1, int(np.ceil(vl[assign[j]].max() / 128.0))) for j in range(BPC)
        )

    qk_np = ml_dtypes.bfloat16 if S_DTYPE == "bf16" else np.float32
    q = np.asarray(queries, dtype=np.float32)
    k = np.asarray(keys, dtype=np.float32)
    v = np.asarray(values, dtype=np.float32)

    in_maps = []
    for c in range(N_CORES):
        bidx = assign[:, c]
        qT = np.ascontiguousarray(q[bidx].transpose(0, 2, 1)).astype(qk_np)
        kT = np.ascontiguousarray(k[bidx].transpose(0, 2, 1)).astype(qk_np)
        mask = (np.arange(K)[None, :] < vl[bidx][:, None]).astype(np.float32)
        vaug = np.concatenate(
            [v[bidx] * mask[:, :, None], mask[:, :, None]], axis=2
        )  # [BPC, K, D+1]
        vaug = np.ascontiguousarray(
            vaug.reshape(BPC, KB_MAX, 128, D + 1)
        ).astype(ml_dtypes.bfloat16)
        in_maps.append({"qT": qT, "kT": kT, "vaug": vaug})
    return in_maps, assign, kb_counts


def kernel(queries, keys, values, valid_length):
    global LAST_RESULTS
    in_maps, assign, kb_counts = _prep(queries, keys, values, valid_length)
    nc = _build(kb_counts, S_DTYPE)
    res = run_bass_kernel_spmd(nc, in_maps, list(range(N_CORES)))
    LAST_RESULTS = res
    out = np.empty((B, Q, D), np.float32)
    for c in range(N_CORES):
        o = np.asarray(res.results[c]["out"], dtype=np.float32)
        for j in range(BPC):
            out[assign[j, c]] = o[j]
    return out
